# revision 1
# baseline (speedup 1.0000x reference)
"""Multi-head attention kernel for Trainium2, 8-core SPMD.

Problem: q,k,v [B=2, H=16, S=2048, D=128] fp32 ->
         softmax(q@k^T/sqrt(D)) @ v, same shape.

Sharding: 32 (b,h) pairs split across 8 cores -> 4 heads per core, each
core computing full attention for its heads independently (no comms).

Per-core pipeline, per head (Q^T/K^T = [d=128, s=2048] via DMA-xbar):
  A-chunks (DMA-heavy): S=Q K^T per q-tile -> ACT exp (+row-sum accum)
    -> DMA-xbar transpose of P -> O^T = sum_j V_j^T P^T_j.
  B-chunks (PE-heavy):  S^T=K Q^T computed directly in [k, q] layout ->
    ACT exp -> P^T with no transpose; row-sums via a ones-vector matmul
    (partition reduction on PE), transposed back to [q,1] with tiny
    PE transposes.
  The A/B mix balances DMA-transpose bandwidth against PE matmul
  throughput. Both chunk kinds share the O^T accumulation, the final
  O^T -> O xbar transpose, the 1/rowsum scaling on DVE and fp32 store.

Emission order software-pipelines chunks explicitly: stage1(c) (scores ->
exp -> P^T) is emitted before stage2(c-1) (O^T matmuls -> output) so the
Tile scheduler (priority ~ program order) always has score-matmul work for
the PE while chunk c-1's P^T transposes drain on the DMA engines.

Sync-wait legality: DMA_DIRECT2D_XPOSE supports a single HW sync-wait
slot; bass_rust.generate_event_semaphores splits multi-wait instructions
into EventSemaphore chains after Tile scheduling.
"""

import numpy as np

import concourse.bass as bass
import concourse.mybir as mybir
import concourse.tile as tile

NCORES = 8
B, H, S, D = 2, 16, 2048, 128
HPC = (B * H) // NCORES  # heads per core = 4
P = 128                  # partitions / tile rows
NT = S // P              # 16 q/k tiles per head
NG = S // 512            # 4 q-chunks of 512
SCALE = 1.0 / float(np.sqrt(D))

# chunk kind per (head, chunk): 'A' = xbar-transposed P, 'B' = transposed-S
CHUNK_KINDS = [
    "BBBB",
    "BBBB",
    "BBBB",
    "BBBB",
]

F32 = mybir.dt.float32
BF16 = mybir.dt.bfloat16
EXP = mybir.ActivationFunctionType.Exp


class _Ctx:
    pass


def _prologue(nc, pools, q, k, v, h, ctx):
    """Loads + casts + Q/K transposes for head h.

    Loads and transposes are issued in quarter-head pieces so the first
    score matmuls only wait ~1/4 of a head's load latency, and so no
    single transfer monopolizes the DMA engines at head boundaries.
    """
    qn = pools["natb"].tile([P, NT, D], BF16, tag="natb")
    kn = pools["natb"].tile([P, NT, D], BF16, tag="natb")
    vn = pools["vn"].tile([P, NT, D], BF16)
    qt = pools["qt"].tile([P, NT, P], BF16)  # qt[d, t, qq] = Q[t*128+qq, d]
    kt = pools["kt"].tile([P, NT, P], BF16)  # kt[d, t, kk] = K[t*128+kk, d]
    kr = k[h].rearrange("(t p) d -> p t d", p=P)
    qr = q[h].rearrange("(t p) d -> p t d", p=P)
    # head 0 is the pipeline ramp: halve its load/transpose pieces so the
    # first score matmuls (which need only the first k/q tiles) start early
    npiece = 1
    step = NT // npiece
    for piece in range(npiece):
        ts = slice(piece * step, (piece + 1) * step)
        nc.gpsimd.dma_start(kn[:, ts, :], kr[:, ts, :])
        nc.gpsimd.dma_start(qn[:, ts, :], qr[:, ts, :])
        nc.sync.dma_start(kt[:, ts, :], kn[:, ts, :], transpose=True)
        nc.sync.dma_start(qt[:, ts, :], qn[:, ts, :], transpose=True)
    vr = v[h].rearrange("(t p) d -> p t d", p=P)
    for piece in range(4):
        ts = slice(piece * 4, (piece + 1) * 4)
        nc.gpsimd.dma_start(vn[:, ts, :], vr[:, ts, :])
    ctx.qt, ctx.kt, ctx.vn = qt, kt, vn


def _stage1(nc, pools, ctx, g, kind, consts):
    """Scores -> exp -> P^T (and, for A, row-sum accum) for chunk g."""
    st = _Ctx()
    st.kind = kind
    st.vn = ctx.vn
    qt, kt = ctx.qt, ctx.kt
    ptg = pools["ptg"].tile([P, NT, 512], BF16)
    st.ptg = ptg

    if kind == "A":
        racc = pools["racc"].tile([P, 8], F32)  # exp sums, col = half*4+li
        st.racc = racc
        for li in range(4):
            qi = g * 4 + li
            pb = pools["pb"].tile([P, S], BF16)
            for half in range(2):
                sp = pools["spsum"].tile([P, 1024], F32)
                for jj in range(2):
                    c = half * 2 + jj
                    nc.tensor.matmul(
                        sp[:, jj * 512:(jj + 1) * 512],
                        lhsT=qt[:, qi, :],
                        rhs=kt[:, c * 4:(c + 1) * 4, :],
                        start=True,
                        stop=True,
                    )
                nc.scalar.activation(
                    pb[:, half * 1024:(half + 1) * 1024],
                    sp[:],
                    EXP,
                    scale=SCALE,
                    accum_out=racc[:, half * 4 + li:half * 4 + li + 1],
                )
            nc.sync.dma_start(
                ptg[:, :, li * P:(li + 1) * P], pb[:], transpose=True
            )
    else:
        # B: S^T = K Q^T computed directly as [k, q] tiles
        for jj in range(NT // 2):
            sp = pools["spsum"].tile([P, 1024], F32)
            for u in range(2):
                j = jj * 2 + u
                nc.tensor.matmul(
                    sp[:, u * 512:(u + 1) * 512],
                    lhsT=kt[:, j, :],
                    rhs=qt[:, g * 4:(g + 1) * 4, :],
                    start=True,
                    stop=True,
                )
            nc.scalar.activation(
                ptg[:, 2 * jj:2 * jj + 2, :], sp[:], EXP, scale=SCALE
            )
    return st


def _stage2(nc, pools, st, o, h, g, consts):
    """Row-sum reciprocal, O^T accumulation, transpose, scale, store."""
    ptg, vn = st.ptg, st.vn
    ones_sb, ident1 = consts

    if st.kind == "A":
        rrec = pools["rr"].tile([P, 4], F32, tag="rrec")
        rsum = pools["rr"].tile([P, 4], F32, tag="rsum")
        nc.vector.tensor_add(rsum[:], st.racc[:, 0:4], st.racc[:, 4:8])
        nc.vector.reciprocal(rrec[:], rsum[:])
    else:
        # row sums r[q] = sum_k P^T[k, q] via ones matmul on PE, then
        # reciprocal and tiny PE transposes back to [q, 1] layout.
        rp = pools["rpsum"].tile([1, 512], F32, tag="rp")
        for j in range(NT):
            nc.tensor.matmul(
                rp[:],
                lhsT=ones_sb[:],
                rhs=ptg[:, j, :],
                start=(j == 0),
                stop=(j == NT - 1),
            )
        r_sb = pools["rr"].tile([1, 512], F32, tag="rb")
        nc.vector.reciprocal(r_sb[:], rp[:])
        rt = pools["rpsum"].tile([P, 4], F32, tag="rt")
        for li in range(4):
            nc.tensor.matmul(
                rt[:, li:li + 1],
                lhsT=r_sb[:, li * P:(li + 1) * P],
                rhs=ident1[:],
                is_transpose=True,
                start=True,
                stop=True,
            )
        rrec = pools["rr"].tile([P, 4], F32, tag="rrec")
        nc.vector.tensor_copy(rrec[:], rt[:])

    ot = pools["otpsum"].tile([P, 512], F32)
    for j in range(NT):
        nc.tensor.matmul(
            ot[:],
            lhsT=vn[:, j, :],
            rhs=ptg[:, j, :],
            start=(j == 0),
            stop=(j == NT - 1),
        )

    otsb = pools["otsb"].tile([P, 512], BF16)
    nc.vector.tensor_copy(otsb[:], ot[:])
    otr = pools["otr"].tile([P, 4, P], BF16)  # otr[qq, li, d] = O[...]
    nc.sync.dma_start(otr[:], otsb[:], transpose=True)

    osb = pools["osb"].tile([P, 4, P], F32)
    nc.vector.tensor_mul(
        osb[:], otr[:], rrec[:, :, None].to_broadcast([P, 4, P])
    )
    nc.gpsimd.dma_start(
        o[h].rearrange("(g t p) d -> g p t d", p=P, t=4)[g], osb[:]
    )


def attention_tiles(tc: "tile.TileContext", q, k, v, o):
    nc = tc.nc
    with (
        tc.tile_pool(name="natb", bufs=4) as natp,
        tc.tile_pool(name="vn", bufs=2) as vnp,
        tc.tile_pool(name="qt", bufs=2) as qtp,
        tc.tile_pool(name="kt", bufs=2) as ktp,
        tc.tile_pool(name="spsum", bufs=2, space="PSUM") as spp,
        tc.tile_pool(name="otpsum", bufs=2, space="PSUM") as otp,
        tc.tile_pool(name="rpsum", bufs=1, space="PSUM") as rpp,
        tc.tile_pool(name="pb", bufs=8) as pbp,
        tc.tile_pool(name="ptg", bufs=4) as ptp,
        tc.tile_pool(name="otsb", bufs=2) as otsbp,
        tc.tile_pool(name="otr", bufs=2) as otrp,
        tc.tile_pool(name="osb", bufs=2) as osbp,
        tc.tile_pool(name="racc", bufs=4) as raccp,
        tc.tile_pool(name="rr", bufs=8) as rrp,
        tc.tile_pool(name="const", bufs=1) as constp,
    ):
        pools = {
            "natb": natp, "vn": vnp, "qt": qtp, "kt": ktp,
            "spsum": spp, "otpsum": otp, "rpsum": rpp,
            "pb": pbp, "ptg": ptp, "otsb": otsbp, "otr": otrp,
            "osb": osbp, "racc": raccp, "rr": rrp,
        }
        ones_sb = constp.tile([P, 1], BF16, tag="ones")
        nc.vector.memset(ones_sb[:], 1.0)
        ident1 = constp.tile([1, 1], F32, tag="ident")
        nc.vector.memset(ident1[:], 1.0)
        consts = (ones_sb, ident1)

        head_ctx = {}
        head_ctx[0] = _Ctx()
        _prologue(nc, pools, q, k, v, 0, head_ctx[0])

        NCHUNK = HPC * NG
        pending = None  # (st, h, g) awaiting stage2
        for ci in range(NCHUNK):
            h, g = divmod(ci, NG)
            if g == 0 and h + 1 < HPC:
                head_ctx[h + 1] = _Ctx()
                _prologue(nc, pools, q, k, v, h + 1, head_ctx[h + 1])
            st = _stage1(nc, pools, head_ctx[h], g, CHUNK_KINDS[h][g], consts)
            if pending is not None:
                _stage2(nc, pools, *pending, consts)
            pending = (st, o, h, g)
        _stage2(nc, pools, *pending, consts)


def build_nc():
    nc = bass.Bass()
    q = nc.declare_dram_parameter("q", [HPC, S, D], F32, isOutput=False)
    k = nc.declare_dram_parameter("k", [HPC, S, D], F32, isOutput=False)
    v = nc.declare_dram_parameter("v", [HPC, S, D], F32, isOutput=False)
    o = nc.declare_dram_parameter("o", [HPC, S, D], F32, isOutput=True)
    with tile.TileContext(nc) as tc:
        attention_tiles(tc, q.ap(), k.ap(), v.ap(), o.ap())
    # Legalize sync waits: DMA_DIRECT2D_XPOSE (and friends) only support a
    # single HW sync-wait slot; this splits multi-wait instructions into
    # EventSemaphore chains (same pass bacc runs for raw-bass kernels).
    import bass_rust

    bass_rust.generate_event_semaphores(nc)
    return nc


_NC_CACHE = None


def get_nc():
    global _NC_CACHE
    if _NC_CACHE is None:
        _NC_CACHE = build_nc()
    return _NC_CACHE


def shard_inputs(q, k, v):
    """Full [B,H,S,D] -> list of per-core input dicts."""
    qf = np.ascontiguousarray(np.asarray(q, dtype=np.float32).reshape(B * H, S, D))
    kf = np.ascontiguousarray(np.asarray(k, dtype=np.float32).reshape(B * H, S, D))
    vf = np.ascontiguousarray(np.asarray(v, dtype=np.float32).reshape(B * H, S, D))
    maps = []
    for c in range(NCORES):
        sl = slice(c * HPC, (c + 1) * HPC)
        maps.append(
            {
                "q": np.ascontiguousarray(qf[sl]),
                "k": np.ascontiguousarray(kf[sl]),
                "v": np.ascontiguousarray(vf[sl]),
            }
        )
    return maps


def unshard_output(results):
    """List of per-core {'o': [HPC,S,D]} -> full [B,H,S,D] fp32."""
    out = np.empty((B * H, S, D), dtype=np.float32)
    for c in range(NCORES):
        out[c * HPC:(c + 1) * HPC] = np.asarray(results[c]["o"], dtype=np.float32)
    return out.reshape(B, H, S, D)


def kernel(q, k, v):
    from concourse.bass_utils import run_bass_kernel_spmd

    nc = get_nc()
    in_maps = shard_inputs(q, k, v)
    res = run_bass_kernel_spmd(nc, in_maps, list(range(NCORES)))
    return unshard_output(res.results)


if __name__ == "__main__":
    rng = np.random.default_rng(0)
    q = rng.standard_normal((B, H, S, D), dtype=np.float32)
    k = rng.standard_normal((B, H, S, D), dtype=np.float32)
    v = rng.standard_normal((B, H, S, D), dtype=np.float32)
    out = kernel(q, k, v)
    print("out", out.shape, out.dtype, float(np.abs(out).max()))

